# revision 11
# baseline (speedup 1.0000x reference)
"""Trainium2 distributed Sinkhorn-EMD loss kernel (nn_CombinedLoss), v2.

Math (per batch element, N=2048 points, D=3):
  C = pairwise euclid(pc1, pc2); K = exp(-C/eps); s = colsum(K)
  50 scale-free Sinkhorn iterations:  u = 1/(K vp);  vp = 1/(K^T u)
  loss = -eps * (R*u) . ((K' o ln K')^T (s o vp));  mean over 16 batches.

Mapping: 2 batch elements per core x 8 cores (data parallel).  K resident
in SBUF in bf16 in BOTH orientations (16 tiles of [128, 2048] each).

Matvec (the hot loop): vector chunk stationary (M=1), matrix tiles stream
through the moving port; FOUR 256-wide output slices run concurrently on
PE column-groups 0/32/64/96 (tile_position col tiling, separate XBUS
streams), two rounds of 4 slices each.  All four groups share ONE PSUM
tile [97, 512] (group g's row at partition 32g; round r at columns
256r:256r+256).  Epilogue per round: three engines (ACT/DVE/Pool)
evacuate the four [1,256] PSUM rows with restriding APs into a
partition-major row buffer [1,128,8], ONE scatter-DMA lands it as
[128,8] f32, and ONE 128-lane DVE reciprocal produces the bf16 vector
half for the next matvec.  This replaces v1's 16 per-column transposing
DMAs + 8 single-lane [1,256] reciprocals per matvec, which serialized at
~10us/matvec.

Finale streams (K'T o ln K'T) chunks (clamp on DVE/Pool, Ln on ACT, mul
split DVE/Pool) through the same 4-group matvec structure, so no second
resident matrix and no 128-column weight loads.
"""

import os
from contextlib import ExitStack

import numpy as np

N = 2048
P = 128
NCH = N // P          # 16 chunks
EPS = 0.01
ITERS = int(os.environ.get("SINK_ITERS", "50"))
NB = 2                # batch elements per core
R = np.float32(1.0 / N)
GROUP = 4             # sqrt/exp table-switch grouping (chunks per group)
PHASES = int(os.environ.get("SINK_PHASES", "3"))  # 1=setup 2=+iters 3=+finale
REPEAT = int(os.environ.get("SINK_REPEAT", "1"))  # benchmark: repeat whole body in-NEFF

_cached = {}


def _build_graph():
    import concourse.bass as bass
    import concourse.mybir as mybir
    import concourse.tile as tile
    from concourse import bacc

    dt = mybir.dt
    AF = mybir.ActivationFunctionType

    nc = bacc.Bacc("TRN2", target_bir_lowering=False, debug=False, num_devices=8)

    # host-packed staging: [NB, 5, 4, N] f32
    #   [:, :, 0] = lhsT rows for [i,j] gram: (-2x0, -2x1, -2x2, x2, 1)
    #   [:, :, 1] = rhs  rows for [i,j] gram: (y0, y1, y2, 1, y2sq)
    #   [:, :, 2] = lhsT rows for [j,i] gram: (-2y0, -2y1, -2y2, y2sq, 1)
    #   [:, :, 3] = rhs  rows for [j,i] gram: (x0, x1, x2, 1, x2)
    stage_d = nc.dram_tensor("stage", [NB, 5, 4, N], dt.float32, kind="ExternalInput").ap()
    out_d = nc.dram_tensor("out", [1, NB], dt.float32, kind="ExternalOutput").ap()

    with tile.TileContext(nc) as tc, ExitStack() as ctx:
        big = ctx.enter_context(tc.tile_pool(name="big", bufs=1))
        cpool = ctx.enter_context(tc.tile_pool(name="cpool", bufs=GROUP))
        fpool = ctx.enter_context(tc.tile_pool(name="fpool", bufs=2))
        small = ctx.enter_context(tc.tile_pool(name="small", bufs=2))
        rowp = ctx.enter_context(tc.tile_pool(name="rowp", bufs=1))
        stgp = ctx.enter_context(tc.tile_pool(name="stgp", bufs=1))
        tpool = ctx.enter_context(tc.tile_pool(name="tpool", bufs=2))
        ypool = ctx.enter_context(tc.tile_pool(name="ypool", bufs=2))
        consts = ctx.enter_context(tc.tile_pool(name="consts", bufs=1))

        ones_f = consts.tile([P, 1], dt.float32, tag="ones_f")
        nc.vector.memset(ones_f, 1.0)
        loss_sb = consts.tile([1, NB], dt.float32, tag="loss_sb")

        # Round structure: 8 output slices of 256 over 3 PE column-groups
        # (0/32/64), rounds of (3,3,2) slices -> y-chunk spans (6,6,4).
        RNDS = [(0, 3), (3, 3), (6, 2)]  # (first slice, n slices)

        def matvec(ips, tiles, x_parts, tag, it):
            """y = 1/(M @ x): stream `tiles` against stationary x chunks.

            x_parts: three tiles [128,6],[128,6],[128,4] (bf16), chunk
            spans 0-5 / 6-11 / 12-15.  Returns (y_parts, t_parts f32)."""
            y_parts, t_parts = [], []
            for r, (s0, ns) in enumerate(RNDS):
                slots = []
                for gi in range(ns):
                    base = 32 * gi
                    pt = (ips.tile([1, 256], dt.float32, tag="psA",
                                   name="psA") if gi == 0 else
                          ips.tile([base + 1, 256], dt.float32,
                                   tag=f"ps{gi}", name=f"ps{gi}"))
                    slots.append((s0 + gi, base, pt[base:base + 1, :]))
                for p in range(NCH):
                    xh = x_parts[min(p // 6, 2)]
                    lhsT = xh[:, p - 6 * min(p // 6, 2):p - 6 * min(p // 6, 2) + 1]
                    for s, base, psl in slots:
                        nc.tensor.matmul(
                            psl, lhsT, tiles[p][:, bass.ds(s * 256, 256)],
                            start=(p == 0), stop=(p == NCH - 1),
                            tile_position=(0, base),
                        )
                # Epilogue: restriding PSUM evacuation (ACT/DVE) -> one
                # partition-major scatter DMA -> one 128-lane reciprocal.
                w = 2 * ns
                row = rowp.tile([1, P, w], dt.float32, tag=f"row{r}",
                                name=f"row_{tag}{r}_{it}")
                for gi, (s, base, psl) in enumerate(slots):
                    src = psl.rearrange("a (c p) -> a p c", c=2, p=P)
                    dst = row[0:1, :, 2 * gi:2 * gi + 2]
                    if gi == 1:
                        nc.vector.tensor_copy(dst, src)
                    else:
                        nc.scalar.copy(dst, src)
                t_part = tpool.tile([P, w], dt.float32, tag=f"t{tag}{r}",
                                    name=f"t_{tag}{r}_{it}")
                (nc.sync if r != 1 else nc.scalar).dma_start(
                    out=t_part, in_=row)
                y_part = ypool.tile([P, w], dt.bfloat16, tag=f"y{tag}{r}",
                                    name=f"y_{tag}{r}_{it}")
                nc.vector.reciprocal(y_part, t_part)
                y_parts.append(y_part)
                t_parts.append(t_part)
            return y_parts, t_parts

        rep_ctx = tc.For_i(0, REPEAT, 1) if REPEAT > 1 else None
        if rep_ctx is not None:
            rep_ctx.__enter__()
        for b in range(NB):
            kt_tiles = [big.tile([P, N], dt.bfloat16, tag=f"kt{p}", name=f"kt{p}_{b}") for p in range(NCH)]
            k_tiles = [big.tile([P, N], dt.bfloat16, tag=f"k{p}", name=f"k{p}_{b}") for p in range(NCH)]
            s_sb = small.tile([P, NCH], dt.float32, tag="s")

            # ---------- setup: build K (both orientations) + column sums ----------
            with tc.tile_pool(name="setup_ps", bufs=3, space="PSUM") as sps:
                for orient in range(2):  # 0 -> [j,i] (KT), 1 -> [i,j] (K)
                    lidx, ridx = (2, 3) if orient == 0 else (0, 1)
                    dest = kt_tiles if orient == 0 else k_tiles
                    stg_l = stgp.tile([5, N], dt.float32, tag="stgl")
                    stg_r = stgp.tile([5, N], dt.float32, tag="stgr")
                    nc.sync.dma_start(out=stg_l, in_=stage_d[b][:, lidx:lidx + 1, :])
                    nc.scalar.dma_start(out=stg_r, in_=stage_d[b][:, ridx:ridx + 1, :])
                    for g0 in range(0, NCH, GROUP):
                        grp = range(g0, min(g0 + GROUP, NCH))
                        ctiles = {}
                        for jc in grp:
                            cfull = cpool.tile([P, N], dt.float32, tag="cfull")
                            ctiles[jc] = cfull
                            for h in range(2):
                                g = sps.tile([P, 1024], dt.float32, tag="gram")
                                for hh in range(2):
                                    nc.tensor.matmul(
                                        g[:, bass.ts(hh, 512)],
                                        stg_l[:, bass.ts(jc, P)],
                                        stg_r[:, bass.ds(h * 1024 + hh * 512, 512)],
                                        start=True, stop=True,
                                    )
                                nc.vector.tensor_scalar_max(g, g, 1e-12)
                                nc.scalar.activation(
                                    cfull[:, bass.ts(h, 1024)], g, AF.Sqrt)
                        for jc in grp:
                            nc.scalar.activation(
                                dest[jc], ctiles[jc], AF.Exp,
                                scale=-1.0 / EPS,
                                accum_out=s_sb[:, jc:jc + 1] if orient == 0 else None,
                            )

            inv_s = small.tile([P, NCH], dt.float32, tag="invs")
            nc.vector.reciprocal(inv_s, s_sb)
            vp_halves = []
            for r, (off, w) in enumerate([(0, 6), (6, 6), (12, 4)]):
                vph = ypool.tile([P, w], dt.bfloat16, tag=f"yv{r}",
                                 name=f"vp0_{r}_{b}")
                nc.vector.tensor_copy(vph, inv_s[:, bass.ds(off, w)])
                vp_halves.append(vph)

            # ---------- 50 Sinkhorn iterations ----------
            t1_halves = t2_halves = None
            with tc.tile_pool(name="iter_ps", bufs=2, space="PSUM") as ips, \
                    nc.allow_low_precision("iterate in bf16"):
                for it in range(ITERS if PHASES >= 2 else 0):
                    u_halves, t1_halves = matvec(ips, kt_tiles, vp_halves, "u", it)
                    vp_halves, t2_halves = matvec(ips, k_tiles, u_halves, "v", it)

            u_f32 = small.tile([P, NCH], dt.float32, tag="uf32")
            v_bf = small.tile([P, NCH], dt.bfloat16, tag="vbf")
            if PHASES >= 2:
                u_r = small.tile([P, NCH], dt.float32, tag="ur")
                v_r = small.tile([P, NCH], dt.float32, tag="vr")
                for r, (off, w) in enumerate([(0, 6), (6, 6), (12, 4)]):
                    nc.vector.reciprocal(u_r[:, bass.ds(off, w)], t1_halves[r])
                    nc.vector.reciprocal(v_r[:, bass.ds(off, w)], t2_halves[r])
                nc.vector.tensor_scalar_mul(u_f32, u_r, float(R))
                v_f = small.tile([P, NCH], dt.float32, tag="vf")
                nc.vector.tensor_mul(v_f, v_r, s_sb)
                nc.vector.tensor_copy(v_bf, v_f)

            if PHASES < 3:
                nc.scalar.activation(
                    loss_sb[0:1, b:b + 1],
                    (s_sb if PHASES == 1 else u_f32)[0:1, b:b + 1],
                    mybir.ActivationFunctionType.Copy, scale=1.0)
                continue

            # ---------- finale: loss = -eps * u . ((K' o ln K')^T v) ----------
            # Streams one KCT chunk at a time through the 4-group matvec
            # structure; clamp and mul split across DVE/Pool, Ln on ACT.
            with tc.tile_pool(name="fin_ps", bufs=2, space="PSUM") as fps:
                mps = fps.tile([97, 512], dt.float32, tag="mps", bufs=1)
                for jc in range(NCH):
                    bp = fpool.tile([P, N], dt.bfloat16, tag="bp",
                                    name=f"bp{jc}_{b}")
                    nc.gpsimd.tensor_scalar_max(
                        bp[:, 0:1024], kt_tiles[jc][:, 0:1024], 1e-38)
                    nc.gpsimd.tensor_scalar_max(
                        bp[:, 1024:2048], kt_tiles[jc][:, 1024:2048], 1e-38)
                    for h in range(2):
                        lnp = fps.tile([P, 1024], dt.float32, tag="lnp")
                        nc.scalar.activation(lnp, bp[:, bass.ts(h, 1024)], AF.Ln)
                        nc.vector.tensor_mul(
                            bp[:, bass.ts(h, 1024)], bp[:, bass.ts(h, 1024)], lnp)
                    for g in range(4):
                        nc.tensor.matmul(
                            mps[32 * g:32 * g + 1, :],
                            v_bf[:, jc:jc + 1],
                            bp[:, bass.ts(g, 512)],
                            start=(jc == 0), stop=(jc == NCH - 1),
                            tile_position=(0, 32 * g),
                        )
                # m row -> partition-major, dot with u, partition-sum.
                row_m = rowp.tile([1, P, NCH], dt.float32, tag="rowm")
                for g in range(4):
                    src = mps[32 * g:32 * g + 1, :].rearrange(
                        "a (c p) -> a p c", c=4, p=P)
                    if g % 2 == 0:
                        nc.scalar.copy(row_m[0:1, :, 4 * g:4 * g + 4], src)
                    else:
                        nc.vector.tensor_copy(row_m[0:1, :, 4 * g:4 * g + 4], src)
                m_pm = small.tile([P, NCH], dt.float32, tag="mpm")
                nc.sync.dma_start(out=m_pm, in_=row_m)
                um = small.tile([P, NCH], dt.float32, tag="um")
                pr = small.tile([P, 1], dt.float32, tag="pr")
                nc.vector.tensor_mul(um, u_f32, m_pm)
                nc.vector.tensor_reduce(
                    pr, um, mybir.AxisListType.X, mybir.AluOpType.add)
                sc_ps = fps.tile([1, 1], dt.float32, tag="sc", bufs=1)
                nc.tensor.matmul(sc_ps, pr, ones_f, start=True, stop=True)
                nc.scalar.activation(
                    loss_sb[0:1, b:b + 1], sc_ps, AF.Copy, scale=-EPS)

        if rep_ctx is not None:
            rep_ctx.__exit__(None, None, None)
        nc.sync.dma_start(out=out_d, in_=loss_sb)

    nc.compile()
    return nc


def _get_graph():
    if "nc" not in _cached:
        _cached["nc"] = _build_graph()
    return _cached["nc"]


def kernel(pc1, pc2, pc3=None, **_unused):
    from concourse.bass_utils import run_bass_kernel_spmd

    x = np.asarray(pc1, dtype=np.float32)
    y = np.asarray(pc2, dtype=np.float32)
    B = x.shape[0]
    x2 = (x * x).sum(-1)
    y2 = (y * y).sum(-1)
    xt = np.moveaxis(x, -1, 1)  # [B, 3, N]
    yt = np.moveaxis(y, -1, 1)

    stage = np.zeros((B, 5, 4, N), np.float32)
    stage[:, 0:3, 0] = -2.0 * xt
    stage[:, 3, 0] = x2
    stage[:, 4, 0] = 1.0
    stage[:, 0:3, 1] = yt
    stage[:, 3, 1] = 1.0
    stage[:, 4, 1] = y2
    stage[:, 0:3, 2] = -2.0 * yt
    stage[:, 3, 2] = y2
    stage[:, 4, 2] = 1.0
    stage[:, 0:3, 3] = xt
    stage[:, 3, 3] = 1.0
    stage[:, 4, 3] = x2

    n_cores = 8
    per = B // n_cores
    assert per == NB, (B, NB)
    in_maps = [
        {"stage": np.ascontiguousarray(stage[c * per:(c + 1) * per])}
        for c in range(n_cores)
    ]
    nc = _get_graph()
    res = run_bass_kernel_spmd(nc, in_maps, list(range(n_cores)))
    losses = np.concatenate([res.results[c]["out"][0] for c in range(n_cores)])
    return np.float32(losses.mean())


# revision 12
# speedup vs baseline: 1.3282x; 1.3282x over previous
"""Trainium2 distributed Sinkhorn-EMD loss kernel (nn_CombinedLoss).

Math (per batch element, N=2048 points, D=3):
  C = pairwise euclid(pc1, pc2); K = exp(-C/eps); s = colsum(K)
  50 Sinkhorn iterations on K_tilde = K/s (folded into the vectors):
      u = r / (K @ (v/s));  v = c*s / (K^T @ u)
  loss = sum(u_i * K_ij * C_ij * v_j);  output = mean over 16 batches.

Mapping: 2 batch elements per core x 8 cores (data parallel).  Per batch,
K is materialized in SBUF in bf16 in BOTH orientations ([i,j] and [j,i])
as 16 tiles of [128, 2048] each, built from a K=5 gram matmul (host
pre-stages [-2x, x^2, 1] rows) -> ACT sqrt -> ACT exp (column sums fall
out of the exp's accum_out).  Matvec structure: the vector chunk is the
STATIONARY operand (M=1) and the matrix tiles stream through the moving
port at 2.4 GHz; two 512-wide output slices run concurrently on PE
column-groups 0 and 32 (tile_position col tiling, separate XBUS streams).
The [1,512] PSUM rows are evacuated by ACT copies and landed
partition-major ([128,16]) by transposing SBUF->SBUF DMAs, hidden behind
the next slices' matmuls; with r == c == 1/N the inter-matvec work
reduces to a single DVE reciprocal per slice.  The finale uses
loss = -eps * u . ((K' o ln K')^T v), reusing the resident KT tiles
(one Ln pass) instead of recomputing C.
"""

import os
from contextlib import ExitStack

import numpy as np

N = 2048
P = 128
NCH = N // P          # 16 chunks
EPS = 0.01
ITERS = int(os.environ.get("SINK_ITERS", "50"))
NB = 2                # batch elements per core
R = np.float32(1.0 / N)
GROUP = 4             # sqrt/exp table-switch grouping (chunks per group)
PHASES = int(os.environ.get("SINK_PHASES", "3"))  # 1=setup 2=+iters 3=+finale
REPEAT = int(os.environ.get("SINK_REPEAT", "1"))  # benchmark: repeat whole body in-NEFF

_cached = {}


def _build_graph():
    import concourse.bass as bass
    import concourse.mybir as mybir
    import concourse.tile as tile
    from concourse import bacc

    dt = mybir.dt
    AF = mybir.ActivationFunctionType

    nc = bacc.Bacc("TRN2", target_bir_lowering=False, debug=False, num_devices=8)

    # host-packed staging: [NB, 5, 4, N] f32
    #   [:, :, 0] = lhsT rows for [i,j] gram: (-2x0, -2x1, -2x2, x2, 1)
    #   [:, :, 1] = rhs  rows for [i,j] gram: (y0, y1, y2, 1, y2sq)
    #   [:, :, 2] = lhsT rows for [j,i] gram: (-2y0, -2y1, -2y2, y2sq, 1)
    #   [:, :, 3] = rhs  rows for [j,i] gram: (x0, x1, x2, 1, x2)
    stage_d = nc.dram_tensor("stage", [NB, 5, 4, N], dt.float32, kind="ExternalInput").ap()
    out_d = nc.dram_tensor("out", [1, NB], dt.float32, kind="ExternalOutput").ap()

    with tile.TileContext(nc) as tc, ExitStack() as ctx:
        big = ctx.enter_context(tc.tile_pool(name="big", bufs=1))
        cpool = ctx.enter_context(tc.tile_pool(name="cpool", bufs=GROUP))
        fpool = ctx.enter_context(tc.tile_pool(name="fpool", bufs=2))
        small = ctx.enter_context(tc.tile_pool(name="small", bufs=2))
        rowp = ctx.enter_context(tc.tile_pool(name="rowp", bufs=4))
        consts = ctx.enter_context(tc.tile_pool(name="consts", bufs=1))

        ones_f = consts.tile([P, 1], dt.float32, tag="ones_f")
        nc.vector.memset(ones_f, 1.0)
        loss_sb = consts.tile([1, NB], dt.float32, tag="loss_sb")

        rep_ctx = tc.For_i(0, REPEAT, 1) if REPEAT > 1 else None
        if rep_ctx is not None:
            rep_ctx.__enter__()
        for b in range(NB):
            stage_sb = big.tile([5, 4, N], dt.float32, tag="stage")
            nc.sync.dma_start(out=stage_sb, in_=stage_d[b])

            kt_tiles = [big.tile([P, N], dt.bfloat16, tag=f"kt{p}", name=f"kt{p}_{b}") for p in range(NCH)]
            k_tiles = [big.tile([P, N], dt.bfloat16, tag=f"k{p}", name=f"k{p}_{b}") for p in range(NCH)]
            s_sb = small.tile([P, NCH], dt.float32, tag="s")

            # ---------- setup: build K (both orientations) + column sums ----------
            with tc.tile_pool(name="setup_ps", bufs=3, space="PSUM") as sps:
                for orient in range(2):  # 0 -> [j,i] (KT), 1 -> [i,j] (K)
                    lidx, ridx = (2, 3) if orient == 0 else (0, 1)
                    dest = kt_tiles if orient == 0 else k_tiles
                    for g0 in range(0, NCH, GROUP):
                        grp = range(g0, min(g0 + GROUP, NCH))
                        ctiles = {}
                        for jc in grp:
                            cfull = cpool.tile([P, N], dt.float32, tag="cfull")
                            ctiles[jc] = cfull
                            for h in range(2):
                                g = sps.tile([P, 1024], dt.float32, tag="gram")
                                for hh in range(2):
                                    nc.tensor.matmul(
                                        g[:, bass.ts(hh, 512)],
                                        stage_sb[:, lidx, bass.ts(jc, P)],
                                        stage_sb[:, ridx, bass.ds(h * 1024 + hh * 512, 512)],
                                        start=True, stop=True,
                                    )
                                nc.vector.tensor_scalar_max(g, g, 1e-12)
                                nc.scalar.activation(
                                    cfull[:, bass.ts(h, 1024)], g, AF.Sqrt)
                        for jc in grp:
                            nc.scalar.activation(
                                dest[jc], ctiles[jc], AF.Exp,
                                scale=-1.0 / EPS,
                                accum_out=s_sb[:, jc:jc + 1] if orient == 0 else None,
                            )

            inv_s = small.tile([P, NCH], dt.float32, tag="invs")
            nc.vector.reciprocal(inv_s, s_sb)
            vp_bf = small.tile([P, NCH], dt.bfloat16, tag="vpbf")
            nc.vector.tensor_copy(vp_bf, inv_s)

            u_f32 = small.tile([P, NCH], dt.float32, tag="uf32")
            v_bf = small.tile([P, NCH], dt.bfloat16, tag="vbf")

            # ---------- 50 Sinkhorn iterations ----------
            # Scale-free (r == c == 1/N cancels): each matvec needs only a
            # reciprocal.  Matvecs run the matrix through the 2.4GHz moving
            # port (lhsT = vector chunk, M=1); the [1,N] PSUM row is copied
            # out by ACT and landed partition-major via transposing DMAs,
            # all hidden behind the next slices' matmuls.
            def matvec(ips, tiles, x_bf, y_bf, want_f32=False):
                # Each [1,512] output slice uses one PE array column, so two
                # slices run concurrently on column-groups 0 and 32, each
                # streaming its own moving operand over a separate XBUS.
                # Epilogue: reciprocal directly on the PSUM row (DVE), then
                # transposing DMAs land it partition-major as the next
                # matvec's bf16 input.  The f32 pre-reciprocal row is only
                # materialized on the last iteration (want_f32).
                t_pm = small.tile([P, NCH], dt.float32, tag="tpm",
                                  name="t_pm")
                SL = 256                      # slice width
                NS = N // SL                  # 8 slices over 3 col-groups
                with nc.allow_low_precision("iterate shadows in bf16"):
                    for r0 in range(0, NS, 3):
                        batch_slices = []
                        for g, s in enumerate(range(r0, min(r0 + 3, NS))):
                            base = 32 * g
                            pt = (ips.tile([1, SL], dt.float32, tag="arA",
                                           name="psA") if g == 0 else
                                  ips.tile([base + 1, SL], dt.float32,
                                           tag=f"ar{g}", name=f"ps{g}"))
                            ps = pt[base:base + 1, :]
                            batch_slices.append((s, base, ps))
                        for p in range(NCH):
                            for s, base, ps in batch_slices:
                                nc.tensor.matmul(
                                    ps,
                                    x_bf[:, p:p + 1],
                                    tiles[p][:, bass.ds(s * SL, SL)],
                                    start=(p == 0), stop=(p == NCH - 1),
                                    tile_position=(0, base),
                                )
                        for s, base, ps in batch_slices:
                            row = rowp.tile([1, SL], dt.bfloat16,
                                            tag="row", name="row")
                            nc.vector.reciprocal(row, ps)
                            for c in range(SL // P):
                                eng = (nc.sync, nc.gpsimd, nc.scalar)[
                                    (s * 2 + c) % 3]
                                eng.dma_start(
                                    out=y_bf[:, s * 2 + c:s * 2 + c + 1],
                                    in_=row[0:1, bass.ds(c * P, P)],
                                )
                            if want_f32:
                                rowf = rowp.tile([1, SL], dt.float32,
                                                 tag="rowf", name="rowf")
                                nc.scalar.copy(rowf, ps)
                                for c in range(SL // P):
                                    nc.sync.dma_start(
                                        out=t_pm[:, s * 2 + c:s * 2 + c + 1],
                                        in_=rowf[0:1, bass.ds(c * P, P)],
                                    )
                return t_pm

            with tc.tile_pool(name="iter_ps", bufs=2, space="PSUM") as ips:
                for it in range(ITERS if PHASES >= 2 else 0):
                    last = it == ITERS - 1
                    u_bf = small.tile([P, NCH], dt.bfloat16, tag="ubf")
                    t1_pm = matvec(ips, kt_tiles, vp_bf, u_bf, want_f32=last)
                    if last:
                        u_r = small.tile([P, NCH], dt.float32, tag="ur")
                        nc.vector.reciprocal(u_r, t1_pm)
                        nc.vector.tensor_scalar_mul(u_f32, u_r, float(R))

                    vp_n = small.tile([P, NCH], dt.bfloat16, tag="vpbf")
                    t2_pm = matvec(ips, k_tiles, u_bf, vp_n, want_f32=last)
                    if last:
                        v_r = small.tile([P, NCH], dt.float32, tag="vr")
                        nc.vector.reciprocal(v_r, t2_pm)
                        nc.vector.tensor_mul(v_bf, v_r, s_sb)
                    vp_bf = vp_n

            if PHASES < 3:
                nc.scalar.activation(
                    loss_sb[0:1, b:b + 1],
                    (s_sb if PHASES == 1 else u_f32)[0:1, b:b + 1],
                    mybir.ActivationFunctionType.Copy, scale=1.0)
                continue

            # ---------- finale: loss = -eps * u . ((K' o ln K')^T v) ----------
            with tc.tile_pool(name="fin_ps", bufs=2, space="PSUM") as fps:
                m_acc = small.tile([P, NCH], dt.float32, tag="macc")
                nc.vector.memset(m_acc, 0.0)
                for jc in range(NCH):
                    bp = big.tile([P, N], dt.bfloat16, tag=f"k{jc}",
                                  name=f"bp{jc}_{b}")
                    nc.vector.tensor_scalar_max(bp, kt_tiles[jc], 1e-38)
                    for h in range(2):
                        lnp = fps.tile([P, 1024], dt.float32, tag="lnp")
                        nc.scalar.activation(lnp, bp[:, bass.ts(h, 1024)], AF.Ln)
                        nc.vector.tensor_mul(
                            bp[:, bass.ts(h, 1024)], bp[:, bass.ts(h, 1024)], lnp)
                    m_ps = fps.tile([P, NCH], dt.float32, tag="m")
                    for q in range(NCH):
                        nc.tensor.matmul(
                            m_ps[:, q:q + 1],
                            bp[:, bass.ts(q, P)],
                            v_bf[:, jc:jc + 1],
                            start=True, stop=True,
                        )
                    nc.vector.tensor_add(m_acc, m_acc, m_ps)
                um = small.tile([P, NCH], dt.float32, tag="um")
                pr = small.tile([P, 1], dt.float32, tag="pr")
                nc.vector.tensor_mul(um, u_f32, m_acc)
                nc.vector.tensor_reduce(
                    pr, um, mybir.AxisListType.X, mybir.AluOpType.add)
                sc_ps = fps.tile([1, 1], dt.float32, tag="sc", bufs=1)
                nc.tensor.matmul(sc_ps, pr, ones_f, start=True, stop=True)
                nc.scalar.activation(
                    loss_sb[0:1, b:b + 1], sc_ps, AF.Copy, scale=-EPS)

        if rep_ctx is not None:
            rep_ctx.__exit__(None, None, None)
        nc.sync.dma_start(out=out_d, in_=loss_sb)

    nc.compile()
    return nc


def _get_graph():
    if "nc" not in _cached:
        _cached["nc"] = _build_graph()
    return _cached["nc"]


def kernel(pc1, pc2, pc3=None, **_unused):
    from concourse.bass_utils import run_bass_kernel_spmd

    x = np.asarray(pc1, dtype=np.float32)
    y = np.asarray(pc2, dtype=np.float32)
    B = x.shape[0]
    x2 = (x * x).sum(-1)
    y2 = (y * y).sum(-1)
    xt = np.moveaxis(x, -1, 1)  # [B, 3, N]
    yt = np.moveaxis(y, -1, 1)

    stage = np.zeros((B, 5, 4, N), np.float32)
    stage[:, 0:3, 0] = -2.0 * xt
    stage[:, 3, 0] = x2
    stage[:, 4, 0] = 1.0
    stage[:, 0:3, 1] = yt
    stage[:, 3, 1] = 1.0
    stage[:, 4, 1] = y2
    stage[:, 0:3, 2] = -2.0 * yt
    stage[:, 3, 2] = y2
    stage[:, 4, 2] = 1.0
    stage[:, 0:3, 3] = xt
    stage[:, 3, 3] = 1.0
    stage[:, 4, 3] = x2

    n_cores = 8
    per = B // n_cores
    assert per == NB, (B, NB)
    in_maps = [
        {"stage": np.ascontiguousarray(stage[c * per:(c + 1) * per])}
        for c in range(n_cores)
    ]
    nc = _get_graph()
    res = run_bass_kernel_spmd(nc, in_maps, list(range(n_cores)))
    losses = np.concatenate([res.results[c]["out"][0] for c in range(n_cores)])
    return np.float32(losses.mean())

